# revision 4
# baseline (speedup 1.0000x reference)
"""Trainium2 Bass kernel for nn_Neuromorphizer (event-camera emulator).

The reference lax.scan collapses exactly to an elementwise op per frame:
with REFRACTORY_US=0 and THRESHOLD=0, `idle` is always true (ts <= t <=
min_time), so state becomes simply the previous frame and timesurface is
dead.  Per frame t (0-indexed), per pixel:

    d    = (tensor[t] - prev) + nb[(t+1) % 10]
    out  = 0 if d < 0, 127 if d == 0, 255 if d > 0

where prev = tensor[t-1] (or `state` for t=0) and nb = +B for on-noise,
-2B for off-noise (off wins), 0 otherwise, with B chosen to dominate any
real |diff| (<= 512).  The classification is computed as
Relu(BIGSCALE*d + 127) followed by a min with 255: exact for d == 0
(0*s+127 = 127) and saturated for |d| >= the minimum representable
nonzero diff (~7.6e-6, scaled far above 255).

Sharding: H=720 rows split across 8 cores (90 rows each); the row shard
of one frame is 90*1280 = 115200 = 128*900 elements -> SBUF tiles of
[128, 900].  No cross-core communication.
"""

import sys

for _p in ("/opt/trn_rl_repo", "/opt/pypackages"):
    if _p not in sys.path:
        sys.path.append(_p)

import numpy as np
import ml_dtypes

import concourse.bacc as bacc
from concourse import mybir
from concourse.tile import TileContext
from concourse.bass_utils import run_bass_kernel_spmd

T, H, W = 96, 720, 1280
N_CORES = 8
ROWS = H // N_CORES          # 90 rows per core
NPIX = ROWS * W              # 115200
P = 128                      # SBUF partitions
FD = NPIX // P               # 900 free-dim elements
N_NOISE = 10

F32 = mybir.dt.float32
BF16 = mybir.dt.bfloat16
U8 = mybir.dt.uint8

# noise bias magnitude: must dominate |cur - prev| <= 512; bf16-exact.
NB_ON = 65536.0
NB_OFF = 131072.0
# scale for the relu trick: smallest nonzero |d| is ~7.6e-6 (f32 grid of
# uniform*255 products); in bf16 >= ~7.4e-6.  7.4e-6 * 1e12 >> 128.
BIGSCALE = 1.0e12

Alu = mybir.AluOpType
Act = mybir.ActivationFunctionType


def build_nc(out_mode: str = "bf16", frames: int = T, fd: int = FD):
    """Build the per-core Bass program.

    out_mode: "f32" (baseline Sign+Relu), "bf16" (bf16 intermediate,
    bf16 output, min on DVE), "u8" (uint8 output via saturating cast).
    """
    nc = bacc.Bacc(debug=False)

    x = nc.dram_tensor("x", [frames, P, fd], F32, kind="ExternalInput")
    st = nc.dram_tensor("state", [P, fd], F32, kind="ExternalInput")
    onm = nc.dram_tensor("on_m", [N_NOISE, P, fd], U8, kind="ExternalInput")
    offm = nc.dram_tensor("off_m", [N_NOISE, P, fd], U8, kind="ExternalInput")
    out_dt = {"f32": F32, "bf16": BF16, "u8": U8}[out_mode]
    y = nc.dram_tensor("y", [frames, P, fd], out_dt, kind="ExternalOutput")

    nb_dt = F32 if out_mode == "f32" else BF16

    with TileContext(nc) as tc:
        with (
            tc.tile_pool(name="const", bufs=1) as cpool,
            tc.tile_pool(name="frames", bufs=6) as fpool,
            tc.tile_pool(name="work", bufs=4) as wpool,
        ):
            # ---- one-time: noise bias planes nb[i] = NB_ON*on - NB_OFF*off
            on_s = cpool.tile([P, N_NOISE * fd], U8, name="on_s")
            off_s = cpool.tile([P, N_NOISE * fd], U8, name="off_s")
            nc.sync.dma_start(
                on_s.rearrange("p (i f) -> p i f", i=N_NOISE),
                onm.rearrange("i p f -> p i f"),
            )
            nc.sync.dma_start(
                off_s.rearrange("p (i f) -> p i f", i=N_NOISE),
                offm.rearrange("i p f -> p i f"),
            )
            bias127 = cpool.tile([P, 1], F32, name="bias127")
            nc.gpsimd.memset(bias127[:], 127.0)

            t_on = cpool.tile([P, N_NOISE * fd], nb_dt, name="t_on")
            nc.scalar.activation(t_on[:], on_s[:], Act.Copy, bias=0.0, scale=NB_ON)
            nb = cpool.tile([P, N_NOISE * fd], nb_dt, name="nb")
            # nb = (off * -NB_OFF) + t_on
            nc.vector.scalar_tensor_tensor(
                nb[:], off_s[:], -NB_OFF, t_on[:], Alu.mult, Alu.add
            )

            # ---- initial prev = state
            prev = fpool.tile([P, fd], F32, name="cur", tag="cur")
            nc.sync.dma_start(prev[:], st[:])

            for t in range(frames):
                nidx = (t + 1) % N_NOISE
                nbv = nb[:, nidx * fd : (nidx + 1) * fd]
                cur = fpool.tile([P, fd], F32, name="cur", tag="cur")
                nc.sync.dma_start(cur[:], x[t])

                if out_mode == "f32":
                    d0 = wpool.tile([P, fd], F32, name="d0")
                    nc.vector.tensor_tensor(d0[:], cur[:], prev[:], Alu.subtract)
                    d = wpool.tile([P, fd], F32, name="d")
                    nc.vector.tensor_tensor(d[:], d0[:], nbv, Alu.add)
                    s = wpool.tile([P, fd], F32, name="s")
                    nc.scalar.activation(s[:], d[:], Act.Sign)
                    out = wpool.tile([P, fd], F32, name="out")
                    nc.scalar.activation(out[:], s[:], Act.Relu, bias=bias127[:], scale=128.0)
                else:
                    d0 = wpool.tile([P, fd], BF16, name="d0")
                    nc.vector.tensor_tensor(d0[:], cur[:], prev[:], Alu.subtract)
                    d = wpool.tile([P, fd], BF16, name="d")
                    nc.vector.tensor_tensor(d[:], d0[:], nbv, Alu.add)
                    r = wpool.tile([P, fd], BF16, name="r")
                    nc.scalar.activation(r[:], d[:], Act.Relu, bias=bias127[:], scale=BIGSCALE)
                    out = wpool.tile([P, fd], out_dt, name="out")
                    nc.vector.tensor_scalar_min(out[:], r[:], 255.0)
                nc.sync.dma_start(y[t], out[:])
                prev = cur
    nc.finalize()
    return nc


_NC_CACHE: dict[str, object] = {}


def _get_nc(out_mode: str):
    if out_mode not in _NC_CACHE:
        _NC_CACHE[out_mode] = build_nc(out_mode)
    return _NC_CACHE[out_mode]


OUT_MODE = "bf16"


def kernel(tensor, state, timesurface=None, on_noise=None, off_noise=None, **_kw):
    tensor = np.asarray(tensor, dtype=np.float32)
    state = np.asarray(state, dtype=np.float32)
    on_u8 = np.ascontiguousarray(on_noise).view(np.uint8)
    off_u8 = np.ascontiguousarray(off_noise).view(np.uint8)

    in_maps = []
    for c in range(N_CORES):
        r0, r1 = c * ROWS, (c + 1) * ROWS
        in_maps.append(
            {
                "x": np.ascontiguousarray(tensor[:, r0:r1, :]).reshape(T, P, FD),
                "state": np.ascontiguousarray(state[r0:r1]).reshape(P, FD),
                "on_m": np.ascontiguousarray(on_u8[:, r0:r1]).reshape(N_NOISE, P, FD),
                "off_m": np.ascontiguousarray(off_u8[:, r0:r1]).reshape(N_NOISE, P, FD),
            }
        )

    nc = _get_nc(OUT_MODE)
    res = run_bass_kernel_spmd(nc, in_maps, core_ids=list(range(N_CORES)))
    shards = [
        np.asarray(res.results[c]["y"]).reshape(T, ROWS, W) for c in range(N_CORES)
    ]
    full = np.concatenate(shards, axis=1)
    return full.astype(np.float32)


# revision 5
# speedup vs baseline: 1.5713x; 1.5713x over previous
"""Trainium2 Bass kernel for nn_Neuromorphizer (event-camera emulator).

The reference lax.scan collapses exactly to an elementwise op per frame:
with REFRACTORY_US=0 and THRESHOLD=0, `idle` is always true (ts <= t <=
min_time), so state becomes simply the previous frame and timesurface is
dead.  Per frame t (0-indexed), per pixel:

    d    = (tensor[t] - prev) + nb[(t+1) % 10]
    out  = 0 if d < 0, 127 if d == 0, 255 if d > 0

where prev = tensor[t-1] (or `state` for t=0) and nb = +B for on-noise,
-2B for off-noise (off wins), 0 otherwise, with B chosen to dominate any
real |diff| (<= 512).  The classification is one ScalarE op,
Relu(BIGSCALE*d + 127), written straight to uint8: the f32->u8 cast
saturates on HW (verified), so d>0 maps to 255, d==0 to exactly 127,
d<0 to 0.  The subtract is exact in f32; its bf16 rounding preserves
sign and zeroness (smallest nonzero |diff| of the uniform*255 grid is
~7.6e-6, far above bf16 underflow).

Sharding: H=720 rows split across 8 cores (90 rows each, no cross-core
communication).  Per-core HBM layout is partition-major [128, T*900] so
a 10-frame chunk DMA moves 36 KB contiguous per partition.  Chunks of
10 frames align exactly with the noise period, so every chunk's noise
bias is the same [128, 10*900] plane buffer (host pre-orders planes by
(idx+1)%10).  Within a chunk the per-frame "previous frame" operand is
the chunk itself shifted one frame; only the chunk's first frame needs
the previous chunk's last frame.
"""

import sys

for _p in ("/opt/trn_rl_repo", "/opt/pypackages"):
    if _p not in sys.path:
        sys.path.append(_p)

import numpy as np

import concourse.bacc as bacc
from concourse import mybir
from concourse.tile import TileContext
from concourse.bass_utils import run_bass_kernel_spmd

T, H, W = 96, 720, 1280
N_CORES = 8
ROWS = H // N_CORES          # 90 rows per core
NPIX = ROWS * W              # 115200
P = 128                      # SBUF partitions
FD = NPIX // P               # 900 free-dim elements per frame
N_NOISE = 10
K = 10                       # frames per chunk (== noise period)

F32 = mybir.dt.float32
BF16 = mybir.dt.bfloat16
U8 = mybir.dt.uint8

NB_ON = 65536.0              # on-noise bias (bf16-exact, dominates |diff|<=512)
NB_OFF = 131072.0            # off-noise bias magnitude (off wins: -2B+B < 0)
BIGSCALE = 1.0e12            # maps smallest nonzero |d| (~7e-6) far above 255

Alu = mybir.AluOpType
Act = mybir.ActivationFunctionType


def build_nc(frames: int = T, fd: int = FD):
    nc = bacc.Bacc(debug=False)

    x = nc.dram_tensor("x", [P, frames * fd], F32, kind="ExternalInput")
    st = nc.dram_tensor("state", [P, fd], F32, kind="ExternalInput")
    onm = nc.dram_tensor("on_m", [N_NOISE, P, fd], U8, kind="ExternalInput")
    offm = nc.dram_tensor("off_m", [N_NOISE, P, fd], U8, kind="ExternalInput")
    y = nc.dram_tensor("y", [P, frames * fd], U8, kind="ExternalOutput")

    chunks = [(c * K, min(K, frames - c * K)) for c in range((frames + K - 1) // K)]

    with TileContext(nc) as tc:
        with (
            tc.tile_pool(name="const", bufs=1) as cpool,
            tc.tile_pool(name="frames", bufs=2) as fpool,
            tc.tile_pool(name="work", bufs=2) as wpool,
        ):
            # ---- one-time: nb[i] = NB_ON*on - NB_OFF*off  (bf16 planes)
            on_s = cpool.tile([P, N_NOISE * fd], U8, name="on_s")
            off_s = cpool.tile([P, N_NOISE * fd], U8, name="off_s")
            nc.sync.dma_start(
                on_s.rearrange("p (i f) -> p i f", i=N_NOISE),
                onm.rearrange("i p f -> p i f"),
            )
            nc.sync.dma_start(
                off_s.rearrange("p (i f) -> p i f", i=N_NOISE),
                offm.rearrange("i p f -> p i f"),
            )
            bias127 = cpool.tile([P, 1], F32, name="bias127")
            nc.gpsimd.memset(bias127[:], 127.0)

            t_on = cpool.tile([P, N_NOISE * fd], BF16, name="t_on")
            nc.scalar.activation(t_on[:], on_s[:], Act.Copy, bias=0.0, scale=NB_ON)
            nb = cpool.tile([P, N_NOISE * fd], BF16, name="nb")
            nc.vector.scalar_tensor_tensor(
                nb[:], off_s[:], -NB_OFF, t_on[:], Alu.mult, Alu.add
            )

            # ---- initial prev(last frame of "chunk -1") = state
            stile = cpool.tile([P, fd], F32, name="stile")
            nc.sync.dma_start(stile[:], st[:])
            prev_last = stile[:, :]

            for f0, k in chunks:
                cfd = k * fd
                cur = fpool.tile([P, K * fd], F32, name="cur", tag="cur")
                nc.sync.dma_start(cur[:, :cfd], x[:, f0 * fd : f0 * fd + cfd])

                d = wpool.tile([P, K * fd], BF16, name="d")
                # frame f0: cur[0] - prev_last ; frames f0+1..: shifted self
                nc.vector.tensor_tensor(
                    d[:, :fd], cur[:, :fd], prev_last, Alu.subtract
                )
                nc.vector.tensor_tensor(
                    d[:, fd:cfd], cur[:, fd:cfd], cur[:, : cfd - fd], Alu.subtract
                )
                # add noise bias in place (bf16 2x)
                nc.vector.tensor_tensor(d[:, :cfd], d[:, :cfd], nb[:, :cfd], Alu.add)
                # classify: saturating u8 cast of Relu(BIGSCALE*d + 127)
                out = wpool.tile([P, K * fd], U8, name="out")
                nc.scalar.activation(
                    out[:, :cfd], d[:, :cfd], Act.Relu, bias=bias127[:], scale=BIGSCALE
                )
                nc.sync.dma_start(y[:, f0 * fd : f0 * fd + cfd], out[:, :cfd])
                prev_last = cur[:, cfd - fd : cfd]
    nc.finalize()
    return nc


_NC_CACHE: dict[str, object] = {}


def _get_nc():
    if "nc" not in _NC_CACHE:
        _NC_CACHE["nc"] = build_nc()
    return _NC_CACHE["nc"]


_NOISE_ORDER = [(i + 1) % N_NOISE for i in range(N_NOISE)]  # storage pos -> plane


def make_in_maps(tensor, state, on_noise, off_noise):
    tensor = np.asarray(tensor, dtype=np.float32)
    state = np.asarray(state, dtype=np.float32)
    on_u8 = np.ascontiguousarray(on_noise).view(np.uint8)[_NOISE_ORDER]
    off_u8 = np.ascontiguousarray(off_noise).view(np.uint8)[_NOISE_ORDER]

    in_maps = []
    for c in range(N_CORES):
        r0, r1 = c * ROWS, (c + 1) * ROWS
        xs = (
            tensor[:, r0:r1, :]
            .reshape(T, P, FD)
            .transpose(1, 0, 2)
            .reshape(P, T * FD)
        )
        in_maps.append(
            {
                "x": np.ascontiguousarray(xs),
                "state": np.ascontiguousarray(state[r0:r1]).reshape(P, FD),
                "on_m": np.ascontiguousarray(on_u8[:, r0:r1]).reshape(N_NOISE, P, FD),
                "off_m": np.ascontiguousarray(off_u8[:, r0:r1]).reshape(
                    N_NOISE, P, FD
                ),
            }
        )
    return in_maps


def gather_output(results):
    shards = []
    for c in range(N_CORES):
        yc = np.asarray(results[c]["y"])  # [P, T*FD] u8
        yc = yc.reshape(P, T, FD).transpose(1, 0, 2).reshape(T, ROWS, W)
        shards.append(yc)
    return np.concatenate(shards, axis=1).astype(np.float32)


def kernel(tensor, state, timesurface=None, on_noise=None, off_noise=None, **_kw):
    in_maps = make_in_maps(tensor, state, on_noise, off_noise)
    nc = _get_nc()
    res = run_bass_kernel_spmd(nc, in_maps, core_ids=list(range(N_CORES)))
    return gather_output(res.results)


# revision 7
# speedup vs baseline: 1.6437x; 1.0461x over previous
"""Trainium2 Bass kernel for nn_Neuromorphizer (event-camera emulator).

The reference lax.scan collapses exactly to an elementwise op per frame:
with REFRACTORY_US=0 and THRESHOLD=0, `idle` is always true (ts <= t <=
min_time), so state becomes simply the previous frame and timesurface is
dead.  Per frame t (0-indexed), per pixel:

    d    = (tensor[t] - prev) + nb[(t+1) % 10]
    out  = 0 if d < 0, 127 if d == 0, 255 if d > 0

where prev = tensor[t-1] (or `state` for t=0) and nb = +B for on-noise,
-2B for off-noise (off wins), 0 otherwise, with B chosen to dominate any
real |diff| (<= 512).  The classification is one ScalarE op,
Relu(BIGSCALE*d + 127), written straight to uint8: the f32->u8 cast
saturates on HW (verified), so d>0 maps to 255, d==0 to exactly 127,
d<0 to 0.  The subtract is exact in f32; its bf16 rounding preserves
sign and zeroness (smallest nonzero |diff| of the uniform*255 grid is
~7.6e-6, far above bf16 underflow).

Sharding: H=720 rows split across 8 cores (90 rows each, no cross-core
communication).  Per-core HBM layout is partition-major [128, T*900] so
a 10-frame chunk DMA moves 36 KB contiguous per partition.  Chunks of
10 frames align exactly with the noise period, so every chunk's noise
bias is the same [128, 10*900] plane buffer (host pre-orders planes by
(idx+1)%10).  Within a chunk the per-frame "previous frame" operand is
the chunk itself shifted one frame; only the chunk's first frame needs
the previous chunk's last frame.
"""

import sys

for _p in ("/opt/trn_rl_repo", "/opt/pypackages"):
    if _p not in sys.path:
        sys.path.append(_p)

import numpy as np

import concourse.bacc as bacc
from concourse import mybir
from concourse.tile import TileContext
from concourse.bass_utils import run_bass_kernel_spmd

T, H, W = 96, 720, 1280
N_CORES = 8
ROWS = H // N_CORES          # 90 rows per core
NPIX = ROWS * W              # 115200
P = 128                      # SBUF partitions
FD = NPIX // P               # 900 free-dim elements per frame
N_NOISE = 10
K = 10                       # frames per chunk (== noise period)

F32 = mybir.dt.float32
BF16 = mybir.dt.bfloat16
U8 = mybir.dt.uint8

NB_ON = 65536.0              # on-noise bias (bf16-exact, dominates |diff|<=512)
NB_OFF = 131072.0            # off-noise bias magnitude (off wins: -2B+B < 0)
BIGSCALE = 1.0e12            # maps smallest nonzero |d| (~7e-6) far above 255

Alu = mybir.AluOpType
Act = mybir.ActivationFunctionType


def build_nc(frames: int = T, fd: int = FD):
    nc = bacc.Bacc(debug=False)

    x = nc.dram_tensor("x", [P, frames * fd], F32, kind="ExternalInput")
    st = nc.dram_tensor("state", [P, fd], F32, kind="ExternalInput")
    nbm = nc.dram_tensor("nb_m", [P, N_NOISE * fd], BF16, kind="ExternalInput")
    y = nc.dram_tensor("y", [P, frames * fd], U8, kind="ExternalOutput")

    chunks = [(c * K, min(K, frames - c * K)) for c in range((frames + K - 1) // K)]

    with TileContext(nc) as tc:
        with (
            tc.tile_pool(name="const", bufs=1) as cpool,
            tc.tile_pool(name="frames", bufs=2) as fpool,
            tc.tile_pool(name="work", bufs=3) as wpool,
        ):
            # first chunk's input load leads the program
            cur0 = fpool.tile([P, K * fd], F32, name="cur", tag="cur")
            nc.sync.dma_start(cur0[:], x[:, : K * fd])

            # constants: host-precomputed noise bias planes, state, bias
            nb = cpool.tile([P, N_NOISE * fd], BF16, name="nb")
            nc.sync.dma_start(nb[:], nbm[:])
            stile = cpool.tile([P, fd], F32, name="stile")
            nc.sync.dma_start(stile[:], st[:])
            bias127 = cpool.tile([P, 1], F32, name="bias127")
            nc.gpsimd.memset(bias127[:], 127.0)

            prev_last = stile[:, :]
            for f0, k in chunks:
                cfd = k * fd
                if f0 == 0:
                    cur = cur0
                else:
                    cur = fpool.tile([P, K * fd], F32, name="cur", tag="cur")
                    nc.sync.dma_start(cur[:, :cfd], x[:, f0 * fd : f0 * fd + cfd])

                d = wpool.tile([P, K * fd], BF16, name="d")
                # frame f0: cur[0] - prev_last ; frames f0+1..: shifted self
                nc.vector.tensor_tensor(
                    d[:, :fd], cur[:, :fd], prev_last, Alu.subtract
                )
                nc.vector.tensor_tensor(
                    d[:, fd:cfd], cur[:, fd:cfd], cur[:, : cfd - fd], Alu.subtract
                )
                # add noise bias in place (bf16 2x)
                nc.vector.tensor_tensor(d[:, :cfd], d[:, :cfd], nb[:, :cfd], Alu.add)
                # classify: saturating u8 cast of Relu(BIGSCALE*d + 127)
                out = wpool.tile([P, K * fd], U8, name="out")
                nc.scalar.activation(
                    out[:, :cfd], d[:, :cfd], Act.Relu, bias=bias127[:], scale=BIGSCALE
                )
                nc.sync.dma_start(y[:, f0 * fd : f0 * fd + cfd], out[:, :cfd])
                prev_last = cur[:, cfd - fd : cfd]
    nc.finalize()
    return nc


_NC_CACHE: dict[str, object] = {}


def _get_nc():
    if "nc" not in _NC_CACHE:
        _NC_CACHE["nc"] = build_nc()
    return _NC_CACHE["nc"]


_NOISE_ORDER = [(i + 1) % N_NOISE for i in range(N_NOISE)]  # storage pos -> plane


def make_nb(on_noise, off_noise):
    """Host-side noise-bias planes: [10, H, W] bf16 in storage order."""
    import ml_dtypes

    on_f = np.asarray(on_noise)[_NOISE_ORDER].astype(np.float32)
    off_f = np.asarray(off_noise)[_NOISE_ORDER].astype(np.float32)
    return (on_f * NB_ON - off_f * NB_OFF).astype(ml_dtypes.bfloat16)


def make_in_maps(tensor, state, on_noise, off_noise):
    tensor = np.asarray(tensor, dtype=np.float32)
    state = np.asarray(state, dtype=np.float32)
    nb = make_nb(on_noise, off_noise)  # [10, H, W] bf16

    in_maps = []
    for c in range(N_CORES):
        r0, r1 = c * ROWS, (c + 1) * ROWS
        xs = (
            tensor[:, r0:r1, :]
            .reshape(T, P, FD)
            .transpose(1, 0, 2)
            .reshape(P, T * FD)
        )
        nbs = (
            nb[:, r0:r1, :]
            .reshape(N_NOISE, P, FD)
            .transpose(1, 0, 2)
            .reshape(P, N_NOISE * FD)
        )
        in_maps.append(
            {
                "x": np.ascontiguousarray(xs),
                "state": np.ascontiguousarray(state[r0:r1]).reshape(P, FD),
                "nb_m": np.ascontiguousarray(nbs),
            }
        )
    return in_maps


def gather_output(results):
    shards = []
    for c in range(N_CORES):
        yc = np.asarray(results[c]["y"])  # [P, T*FD] u8
        yc = yc.reshape(P, T, FD).transpose(1, 0, 2).reshape(T, ROWS, W)
        shards.append(yc)
    return np.concatenate(shards, axis=1).astype(np.float32)


def kernel(tensor, state, timesurface=None, on_noise=None, off_noise=None, **_kw):
    in_maps = make_in_maps(tensor, state, on_noise, off_noise)
    nc = _get_nc()
    res = run_bass_kernel_spmd(nc, in_maps, core_ids=list(range(N_CORES)))
    return gather_output(res.results)


# revision 8
# speedup vs baseline: 1.8095x; 1.1008x over previous
"""Trainium2 Bass kernel for nn_Neuromorphizer (event-camera emulator).

The reference lax.scan collapses exactly to an elementwise op per frame:
with REFRACTORY_US=0 and THRESHOLD=0, `idle` is always true (ts <= t <=
min_time), so state becomes simply the previous frame and timesurface is
dead.  Per frame t (0-indexed), per pixel:

    d    = (tensor[t] - prev) + nb[(t+1) % 10]
    out  = 0 if d < 0, 127 if d == 0, 255 if d > 0

where prev = tensor[t-1] (or `state` for t=0) and nb = +B for on-noise,
-2B for off-noise (off wins), 0 otherwise, with B chosen to dominate any
real |diff| (<= 512).  The classification is one ScalarE op,
Relu(BIGSCALE*d + 127), written straight to uint8: the f32->u8 cast
saturates on HW (verified), so d>0 maps to 255, d==0 to exactly 127,
d<0 to 0.  The subtract is exact in f32; its bf16 rounding preserves
sign and zeroness (smallest nonzero |diff| of the uniform*255 grid is
~7.6e-6, far above bf16 underflow).

Sharding: H=720 rows split across 8 cores (90 rows each, no cross-core
communication).  Per-core HBM layout is partition-major [128, T*900] so
a 10-frame chunk DMA moves 36 KB contiguous per partition.  Chunks of
10 frames align exactly with the noise period, so every chunk's noise
bias is the same [128, 10*900] plane buffer (host pre-orders planes by
(idx+1)%10).  Within a chunk the per-frame "previous frame" operand is
the chunk itself shifted one frame; only the chunk's first frame needs
the previous chunk's last frame.
"""

import sys

for _p in ("/opt/trn_rl_repo", "/opt/pypackages"):
    if _p not in sys.path:
        sys.path.append(_p)

import numpy as np

import concourse.bacc as bacc
from concourse import mybir
from concourse.tile import TileContext
from concourse.bass_utils import run_bass_kernel_spmd

T, H, W = 96, 720, 1280
N_CORES = 8
ROWS = H // N_CORES          # 90 rows per core
NPIX = ROWS * W              # 115200
P = 128                      # SBUF partitions
FD = NPIX // P               # 900 free-dim elements per frame
N_NOISE = 10
K = 10                       # frames per chunk (== noise period)

F32 = mybir.dt.float32
BF16 = mybir.dt.bfloat16
U8 = mybir.dt.uint8

NB_ON = 65536.0              # on-noise bias (bf16-exact, dominates |diff|<=512)
NB_OFF = 131072.0            # off-noise bias magnitude (off wins: -2B+B < 0)
BIGSCALE = 1.0e12            # maps smallest nonzero |d| (~7e-6) far above 255

Alu = mybir.AluOpType
Act = mybir.ActivationFunctionType


def build_nc(frames: int = T, fd: int = FD):
    nc = bacc.Bacc(debug=False)

    x = nc.dram_tensor("x", [P, frames * fd], F32, kind="ExternalInput")
    st = nc.dram_tensor("state", [P, fd], F32, kind="ExternalInput")
    nbm = nc.dram_tensor("nb_m", [P, N_NOISE * fd], BF16, kind="ExternalInput")
    y = nc.dram_tensor("y", [P, frames * fd], U8, kind="ExternalOutput")

    chunks = [(c * K, min(K, frames - c * K)) for c in range((frames + K - 1) // K)]

    with TileContext(nc) as tc:
        with (
            tc.tile_pool(name="const", bufs=1) as cpool,
            tc.tile_pool(name="frames", bufs=2) as fpool,
            tc.tile_pool(name="work", bufs=3) as wpool,
        ):
            # first chunk's input load (split in halves) leads the program
            cur0 = fpool.tile([P, K * fd], F32, name="cur", tag="cur")
            h0 = (K // 2) * fd
            nc.sync.dma_start(cur0[:, :h0], x[:, :h0])
            nc.sync.dma_start(cur0[:, h0:], x[:, h0 : K * fd])

            # constants: host-precomputed noise bias planes, state, bias
            nb = cpool.tile([P, N_NOISE * fd], BF16, name="nb")
            nc.sync.dma_start(nb[:], nbm[:])
            stile = cpool.tile([P, fd], F32, name="stile")
            nc.sync.dma_start(stile[:], st[:])
            bias127 = cpool.tile([P, 1], F32, name="bias127")
            nc.gpsimd.memset(bias127[:], 127.0)

            prev_last = stile[:, :]
            for f0, k in chunks:
                cfd = k * fd
                half = (k // 2) * fd  # columns in the first half
                if f0 == 0:
                    cur = cur0
                else:
                    cur = fpool.tile([P, K * fd], F32, name="cur", tag="cur")
                    nc.sync.dma_start(
                        cur[:, :half], x[:, f0 * fd : f0 * fd + half]
                    )
                    nc.sync.dma_start(
                        cur[:, half:cfd], x[:, f0 * fd + half : f0 * fd + cfd]
                    )

                d = wpool.tile([P, K * fd], BF16, name="d")
                # frame f0: cur[0] - prev_last ; rest: shifted self, split so
                # each piece depends on only one input half-DMA
                nc.vector.tensor_tensor(
                    d[:, :fd], cur[:, :fd], prev_last, Alu.subtract
                )
                nc.vector.tensor_tensor(
                    d[:, fd:half], cur[:, fd:half], cur[:, : half - fd], Alu.subtract
                )
                nc.vector.tensor_tensor(
                    d[:, half:cfd],
                    cur[:, half:cfd],
                    cur[:, half - fd : cfd - fd],
                    Alu.subtract,
                )
                out = wpool.tile([P, K * fd], U8, name="out")
                for a, b in ((0, half), (half, cfd)):
                    # add noise bias in place (bf16 2x), then classify via
                    # saturating u8 cast of Relu(BIGSCALE*d + 127)
                    nc.vector.tensor_tensor(
                        d[:, a:b], d[:, a:b], nb[:, a:b], Alu.add
                    )
                    nc.scalar.activation(
                        out[:, a:b], d[:, a:b], Act.Relu, bias=bias127[:],
                        scale=BIGSCALE,
                    )
                nc.sync.dma_start(y[:, f0 * fd : f0 * fd + cfd], out[:, :cfd])
                prev_last = cur[:, cfd - fd : cfd]
    nc.finalize()
    return nc


_NC_CACHE: dict[str, object] = {}


def _get_nc():
    if "nc" not in _NC_CACHE:
        _NC_CACHE["nc"] = build_nc()
    return _NC_CACHE["nc"]


_NOISE_ORDER = [(i + 1) % N_NOISE for i in range(N_NOISE)]  # storage pos -> plane


def make_nb(on_noise, off_noise):
    """Host-side noise-bias planes: [10, H, W] bf16 in storage order."""
    import ml_dtypes

    on_f = np.asarray(on_noise)[_NOISE_ORDER].astype(np.float32)
    off_f = np.asarray(off_noise)[_NOISE_ORDER].astype(np.float32)
    return (on_f * NB_ON - off_f * NB_OFF).astype(ml_dtypes.bfloat16)


def make_in_maps(tensor, state, on_noise, off_noise):
    tensor = np.asarray(tensor, dtype=np.float32)
    state = np.asarray(state, dtype=np.float32)
    nb = make_nb(on_noise, off_noise)  # [10, H, W] bf16

    in_maps = []
    for c in range(N_CORES):
        r0, r1 = c * ROWS, (c + 1) * ROWS
        xs = (
            tensor[:, r0:r1, :]
            .reshape(T, P, FD)
            .transpose(1, 0, 2)
            .reshape(P, T * FD)
        )
        nbs = (
            nb[:, r0:r1, :]
            .reshape(N_NOISE, P, FD)
            .transpose(1, 0, 2)
            .reshape(P, N_NOISE * FD)
        )
        in_maps.append(
            {
                "x": np.ascontiguousarray(xs),
                "state": np.ascontiguousarray(state[r0:r1]).reshape(P, FD),
                "nb_m": np.ascontiguousarray(nbs),
            }
        )
    return in_maps


def gather_output(results):
    shards = []
    for c in range(N_CORES):
        yc = np.asarray(results[c]["y"])  # [P, T*FD] u8
        yc = yc.reshape(P, T, FD).transpose(1, 0, 2).reshape(T, ROWS, W)
        shards.append(yc)
    return np.concatenate(shards, axis=1).astype(np.float32)


def kernel(tensor, state, timesurface=None, on_noise=None, off_noise=None, **_kw):
    in_maps = make_in_maps(tensor, state, on_noise, off_noise)
    nc = _get_nc()
    res = run_bass_kernel_spmd(nc, in_maps, core_ids=list(range(N_CORES)))
    return gather_output(res.results)
